# revision 24
# baseline (speedup 1.0000x reference)
"""AttentionPooling (global-softmax segment-sum) Trainium2 Bass kernel.

  scores = x @ W + b ; attn = softmax(scores, axis=0) ; out = segment_sum(x*attn, batch, G)

Design (8 cores, SPMD, raw Bass; softmax is shift-invariant so b drops out and
the fixed shift is 0; device computes unnormalized pooled sums + Z partials,
host divides at the end):

 * Segments are sorted by size (desc) and snake-dealt to the 8 cores, so every
   core sees a near-identical segment-size profile (cumulative node drift
   between cores < 1 chunk).  That allows ONE shared SPMD program in which
   chunk c of every core covers segments inside a shared window
   [W0(c), W0(c)+K) with small K (~4): the segment-scatter matrix per chunk is
   only [128, K] instead of a full [128, 128] one-hot.
 * x ships TRANSPOSED per 128-node chunk: xT_c [d=128 part, n=128 free] bf16,
   packed in 32-chunk DMA slabs (8 KB/partition lines -> full DMA efficiency).
 * PE per chunk (matmul operands in SBUF):
     scores:   mm(lhsT=xT_c, rhs=W[d,1])   -> psum col  [n,1] f32   (~2 ns)
     untrans:  PE transpose(xT_c)          -> psum x_c [n,d] BF16   (~53 ns)
     pooled:   mm(lhsT=x_c(sbuf), rhs=M_c[n,K]) += psum out[d, segcols]
               (start=False, banks double-buffered)                 (~2 ns)
   The pooled output lands TRANSPOSED [d, seg]; the host untransposes.
 * bf16 psum transposes pack 8 chunks per bank, so the psum->sbuf copies are
   [128, 1024] bf16 ops (2x mode on DVE) split across DVE and ACT.
 * ACT: Exp on 32-wide score strips (psum f32 -> sbuf expw f32).
 * DVE/GP: masks M_c = (iota_K == bl_c) * expw_c via one tensor_scalar
   [128,K] bf16 per chunk (~61/99 ns).  Z = one tensor_reduce over expw
   [128, nch] at the very end.

PSUM hazard rule (found the hard way; the device hangs otherwise): a bank PE
is writing must not be concurrently accessed by ACT/DVE.  Hence: scores
alternate between 2 banks per 32-chunk strip and PE re-enters a parity only
after that parity's previous Exp finished; transposed x rotates 4 banks
(copies read banks PE is not writing); the out accumulators are 2 banks
double-buffered over the (chunk-sequential) 512-segment ranges with
flush+memset strictly between PE uses.

TimelineSim (the graded cost model) is DMA-bound: the 65 MB/core bf16 x
stream at the modeled 360 GB/s is ~184 us.
"""

import hashlib
import os
import numpy as np
import ml_dtypes

import concourse.bass as bass
import concourse.mybir as mybir
from concourse.bass_utils import run_bass_kernel_spmd

BF16 = mybir.dt.bfloat16
F32 = mybir.dt.float32
ALU = mybir.AluOpType
ACTF = mybir.ActivationFunctionType

N_CORES = 8
P = 128
D = 128
SUP_CH = 32          # chunks per DMA super-slab
GRP = 8              # chunks per transpose-psum bank / copy op
NXB = 10             # xT slab ring depth
NT = 4               # transpose psum bank rotation
NXS = 12             # copied-back x_c sbuf slots (GRP-chunk groups)
NM = 128             # mask sbuf slots
LAG_G = 9            # pooled mms lag transposes by this many GRP-groups
BATCHW = 32          # pooled mask-batch wait granularity (chunks)
EXPW = 32            # chunks per Exp strip
MLAG = 16            # exp/masks lag copies by this many chunks
KCAP = 16            # pass-1 span cap

_prog_cache = {}


def _build(nch, K, n_banks, bank_of, jb_of, sup_sizes, mask_dve, copy_eng):
    """Shared SPMD program.  bank_of/jb_of: per-chunk out range and column
    base.  sup_sizes: chunks per DMA super.  mask_dve[c]: mask built on DVE
    (else GPSIMD).  copy_eng[g]: 0=DVE 1=ACT for GRP-chunk psum->sbuf copies."""
    nsup = len(sup_sizes)
    CH0 = [0]
    for t in sup_sizes:
        CH0.append(CH0[-1] + t)
    assert CH0[-1] == nch and nch % GRP == 0
    ngrp = nch // GRP
    n_exp = (nch + EXPW - 1) // EXPW
    sup_of = []
    for s in range(nsup):
        sup_of += [s] * sup_sizes[s]

    # cumulative ticks
    mskd_tick = np.cumsum(mask_dve).tolist()
    mskg_tick = np.cumsum([not m for m in mask_dve]).tolist()
    cp_tick = [0] * ngrp
    cnt = [0, 0]
    for g in range(ngrp):
        cnt[copy_eng[g]] += 1
        cp_tick[g] = cnt[copy_eng[g]]

    G0 = [0]
    for c in range(nch):
        if bank_of[c] != len(G0) - 1:
            G0.append(c)
    G0 += [nch] * (n_banks + 1 - len(G0))

    nc = bass.Bass()
    xp_h = nc.declare_dram_parameter("xp", [nch * P * D], BF16, isOutput=False)
    bl_h = nc.declare_dram_parameter("bl", [P, nch], F32, isOutput=False)
    wcol_h = nc.declare_dram_parameter("wcol", [P, 1], BF16, isOutput=False)
    ident_h = nc.declare_dram_parameter("ident", [P, P], BF16, isOutput=False)
    iota_h = nc.declare_dram_parameter("iota", [P, K], BF16, isOutput=False)
    out_h = nc.declare_dram_parameter("outp", [P, n_banks * 512], F32, isOutput=True)
    z_h = nc.declare_dram_parameter("zout", [P, 1], F32, isOutput=True)

    import contextlib
    with contextlib.ExitStack() as ctx:
        sem_x = [ctx.enter_context(nc.semaphore(f"sem_x{j}")) for j in range(NXB)]
        sem_cst = ctx.enter_context(nc.semaphore("sem_cst"))
        sem_sc = ctx.enter_context(nc.semaphore("sem_sc"))
        sem_tr = ctx.enter_context(nc.semaphore("sem_tr"))
        sem_ex = ctx.enter_context(nc.semaphore("sem_ex"))
        sem_md = ctx.enter_context(nc.semaphore("sem_md"))
        sem_mg = ctx.enter_context(nc.semaphore("sem_mg"))
        sem_pl = ctx.enter_context(nc.semaphore("sem_pl"))
        sem_cp = [ctx.enter_context(nc.semaphore(f"sem_cp{e}")) for e in range(2)]
        sem_ini = ctx.enter_context(nc.semaphore("sem_ini"))
        sem_zr = ctx.enter_context(nc.semaphore("sem_zr"))
        sem_fl = ctx.enter_context(nc.semaphore("sem_fl"))
        sem_out = ctx.enter_context(nc.semaphore("sem_out"))

        xt = [ctx.enter_context(nc.sbuf_tensor(f"xt{j}", [P, SUP_CH * D], BF16))
              for j in range(NXB)]
        wcol_t = ctx.enter_context(nc.sbuf_tensor("wcol_t", [P, 1], BF16))
        ident_t = ctx.enter_context(nc.sbuf_tensor("ident_t", [P, P], BF16))
        iota_t = ctx.enter_context(nc.sbuf_tensor("iota_t", [P, K], BF16))
        bl_t = ctx.enter_context(nc.sbuf_tensor("bl_t", [P, nch], F32))
        expw_t = ctx.enter_context(nc.sbuf_tensor("expw_t", [P, nch], F32))
        xsb = [ctx.enter_context(nc.sbuf_tensor(f"xsb{j}", [P, GRP * D], BF16))
               for j in range(NXS)]
        msk = [ctx.enter_context(nc.sbuf_tensor(f"msk{j}", [P, K], BF16))
               for j in range(NM)]
        stage_t = ctx.enter_context(nc.sbuf_tensor("stage_t", [P, n_banks * 512], F32))
        zsum_t = ctx.enter_context(nc.sbuf_tensor("zsum_t", [P, 1], F32))

        # PSUM hazard rule: a bank PE is writing must never be concurrently
        # accessed by ACT/DVE (the device hangs).  Scores: 2 banks alternated
        # per strip; transposes: 4 bf16 banks of GRP chunks; out: 2 banks
        # double-buffered over the sequential 512-seg ranges.
        sp2 = [ctx.enter_context(nc.psum_tensor(f"sp{i}", [P, 512], F32))
               for i in range(2)]
        tp = [ctx.enter_context(nc.psum_tensor(f"tp{j}", [P, GRP * D], BF16))
              for j in range(NT)]
        outp2 = [ctx.enter_context(nc.psum_tensor(f"op{b}", [P, 512], F32))
                 for b in range(2)]

        N_CST = 4  # preamble DMAs

        def sploc(c):
            e = c // EXPW
            return sp2[e % 2], ((e // 2) * EXPW) % 512 + (c % EXPW)

        def pooled_group(tensor, go, tail):
            c0 = GRP * go
            if c0 % BATCHW == 0:
                ce = min(c0 + BATCHW, nch) - 1
                tensor.wait_ge(sem_md, mskd_tick[ce])
                tensor.wait_ge(sem_mg, mskg_tick[ce])
            if tail and go >= ngrp - NT:
                tensor.wait_ge(sem_cp[copy_eng[go]], cp_tick[go])
            for cc in range(c0, c0 + GRP):
                r = bank_of[cc]
                if r >= 2 and cc == G0[r]:
                    tensor.wait_ge(sem_ini, r + 1)   # memset of reused bank
                nc.tensor.matmul(
                    outp2[r % 2][:, jb_of[cc]:jb_of[cc] + K],
                    lhsT=xsb[go % NXS][:, (cc % GRP) * D:(cc % GRP + 1) * D],
                    rhs=msk[cc % NM][:],
                    start=False, stop=True, skip_group_check=True,
                ).then_inc(sem_pl, 1)

        with nc.Block() as block:

            @block.sync
            def _(sync):
                sync.dma_start(out=wcol_t[:], in_=wcol_h[:]).then_inc(sem_cst, 16)
                sync.dma_start(out=ident_t[:], in_=ident_h[:]).then_inc(sem_cst, 16)
                sync.dma_start(out=iota_t[:], in_=iota_h[:]).then_inc(sem_cst, 16)
                sync.dma_start(out=bl_t[:], in_=bl_h[:]).then_inc(sem_cst, 16)
                for s in range(nsup):
                    j = s % NXB
                    ch = sup_sizes[s]
                    if s >= NXB:
                        sync.wait_ge(sem_tr, CH0[s - NXB + 1])
                    sync.dma_start(
                        out=xt[j][:, 0:ch * D],
                        in_=xp_h[CH0[s] * P * D:CH0[s + 1] * P * D].rearrange(
                            "(d f) -> d f", d=P),
                    ).then_inc(sem_x[j], 16)
                for b in range(n_banks):
                    sync.wait_ge(sem_fl, b + 1)
                    sync.dma_start(
                        out=out_h[:, b * 512:(b + 1) * 512],
                        in_=stage_t[:, b * 512:(b + 1) * 512],
                    ).then_inc(sem_out, 16)
                sync.wait_ge(sem_zr, 1)
                sync.dma_start(out=z_h[:], in_=zsum_t[:]).then_inc(sem_out, 16)
                sync.wait_ge(sem_out, 16 * (n_banks + 1))

            @block.tensor
            def _(tensor):
                tensor.wait_ge(sem_cst, 16 * N_CST)
                tensor.wait_ge(sem_ini, 2)
                for c in range(nch):
                    s = sup_of[c]
                    ci = c - CH0[s]
                    if ci == 0:
                        tensor.wait_ge(sem_x[s % NXB], 16 * (s // NXB + 1))
                    if c % EXPW == 0 and c // EXPW >= 2:
                        # reuse of this parity's score bank: prior strip's Exp
                        tensor.wait_ge(sem_ex, c // EXPW - 1)
                    xsl = xt[s % NXB][:, ci * D:(ci + 1) * D]
                    bnk, col = sploc(c)
                    nc.tensor.matmul(
                        bnk[:, col:col + 1],
                        lhsT=xsl, rhs=wcol_t[:],
                        start=True, stop=True, skip_group_check=True,
                    ).then_inc(sem_sc, 1)
                    g = c // GRP
                    if c % GRP == 0 and g >= NT:
                        go2 = g - NT
                        tensor.wait_ge(sem_cp[copy_eng[go2]], cp_tick[go2])
                    nc.tensor.transpose(
                        tp[g % NT][:, (c % GRP) * D:(c % GRP + 1) * D],
                        xsl, ident_t[:],
                    ).then_inc(sem_tr, 1)
                    if c % GRP == GRP - 1 and g >= LAG_G:
                        pooled_group(tensor, g - LAG_G, False)
                for go in range(max(0, ngrp - LAG_G), ngrp):
                    pooled_group(tensor, go, True)

            # Copies run at position p; exp/masks trail at p-MLAG so neither
            # ACT nor DVE blocks on exp before emitting a copy PE waits on.

            @block.scalar
            def _(scalar):
                nfl = 0
                for p in range(0, nch + MLAG, GRP):
                    g = p // GRP
                    if g < ngrp and copy_eng[g] == 1:
                        scalar.wait_ge(sem_tr, GRP * g + GRP)
                        if g >= NXS:
                            scalar.wait_ge(sem_pl, GRP * (g - NXS) + GRP)
                        nc.scalar.copy(
                            out=xsb[g % NXS][:], in_=tp[g % NT][:],
                        ).then_inc(sem_cp[1], 1)
                    cm = p - MLAG
                    if cm >= 0 and cm % EXPW == 0:
                        e = cm // EXPW
                        c0, c1 = EXPW * e, min(EXPW * e + EXPW, nch)
                        scalar.wait_ge(sem_sc, c1)
                        bnk, col = sploc(c0)
                        nc.scalar.activation(
                            out=expw_t[:, c0:c1],
                            in_=bnk[:, col:col + (c1 - c0)],
                            func=ACTF.Exp,
                        ).then_inc(sem_ex, 1)
                    while nfl < n_banks and G0[nfl + 1] + 80 <= p:
                        scalar.wait_ge(sem_pl, G0[nfl + 1])
                        nc.scalar.copy(
                            out=stage_t[:, nfl * 512:(nfl + 1) * 512],
                            in_=outp2[nfl % 2][:],
                        ).then_inc(sem_fl, 1)
                        nfl += 1
                while nfl < n_banks:
                    scalar.wait_ge(sem_pl, G0[nfl + 1])
                    nc.scalar.copy(
                        out=stage_t[:, nfl * 512:(nfl + 1) * 512],
                        in_=outp2[nfl % 2][:],
                    ).then_inc(sem_fl, 1)
                    nfl += 1

            @block.vector
            def _(vector):
                for b in range(2):
                    nc.vector.memset(outp2[b][:], 0.0).then_inc(sem_ini, 1)
                vector.wait_ge(sem_cst, 16 * N_CST)
                nms = 2
                for p in range(nch + MLAG):
                    g = p // GRP
                    if p < nch and p % GRP == GRP - 1 and copy_eng[g] == 0:
                        vector.wait_ge(sem_tr, GRP * g + GRP)
                        if g >= NXS:
                            vector.wait_ge(sem_pl, GRP * (g - NXS) + GRP)
                        nc.vector.tensor_copy(
                            out=xsb[g % NXS][:], in_=tp[g % NT][:],
                        ).then_inc(sem_cp[0], 1)
                    while nms < n_banks and G0[nms - 1] + 96 <= p:
                        vector.wait_ge(sem_fl, nms - 1)
                        nc.vector.memset(outp2[nms % 2][:], 0.0).then_inc(sem_ini, 1)
                        nms += 1
                    cm = p - MLAG
                    if cm < 0:
                        continue
                    if cm % EXPW == 0:
                        vector.wait_ge(sem_ex, cm // EXPW + 1)
                        if cm >= NM:
                            vector.wait_ge(sem_pl, cm - NM + 1)
                    if mask_dve[cm]:
                        nc.vector.tensor_scalar(
                            msk[cm % NM][:], iota_t[:],
                            bl_t[:, cm:cm + 1], expw_t[:, cm:cm + 1],
                            ALU.is_equal, ALU.mult,
                        ).then_inc(sem_md, 1)
                # Z = sum over all chunks of expw (pads contribute e^0=1
                # each; host subtracts the pad count)
                vector.wait_ge(sem_ex, n_exp)
                nc.vector.tensor_reduce(
                    out=zsum_t[:], in_=expw_t[:],
                    axis=mybir.AxisListType.X, op=ALU.add,
                ).then_inc(sem_zr, 1)

            @block.gpsimd
            def _(gpsimd):
                # GPSIMD cannot access PSUM: masks only
                gpsimd.wait_ge(sem_cst, 16 * N_CST)
                for cm in range(nch):
                    if cm % EXPW == 0:
                        gpsimd.wait_ge(sem_ex, cm // EXPW + 1)
                        if cm >= NM:
                            gpsimd.wait_ge(sem_pl, cm - NM + 1)
                    if not mask_dve[cm]:
                        nc.gpsimd.tensor_scalar(
                            msk[cm % NM][:], iota_t[:],
                            bl_t[:, cm:cm + 1], expw_t[:, cm:cm + 1],
                            ALU.is_equal, ALU.mult,
                        ).then_inc(sem_mg, 1)

    return nc


def _plan(counts_k, n_banks):
    """Pass-1 chunking for one core: counts_k[j] = node count of local seg j.
    Returns per-group chunk lists [(jf, [(j, off, take), ...]), ...]."""
    groups = []
    nsegs = len(counts_k)
    for gb in range(n_banks):
        glo, ghi = 512 * gb, min(512 * (gb + 1), nsegs)
        chunks = []
        cur_nodes, cur_jf, cur_n = [], None, 0
        for j in range(glo, ghi):
            cnt = int(counts_k[j])
            off = 0
            while cnt > 0:
                if cur_jf is not None and j - cur_jf + 1 > KCAP:
                    chunks.append((cur_jf, cur_nodes))
                    cur_nodes, cur_jf, cur_n = [], None, 0
                if cur_jf is None:
                    cur_jf = j
                take = min(cnt, P - cur_n)
                cur_nodes.append((j, off, take))
                cur_n += take
                off += take
                cnt -= take
                if cur_n == P:
                    chunks.append((cur_jf, cur_nodes))
                    cur_nodes, cur_jf, cur_n = [], None, 0
        if cur_n > 0:
            chunks.append((cur_jf, cur_nodes))
        groups.append(chunks)
    return groups


def _pool(x, batch, W, num_graphs, n_cores=N_CORES):
    segs_per_core = num_graphs // n_cores
    n_banks = (segs_per_core + 511) // 512

    counts = np.bincount(batch, minlength=num_graphs).astype(np.int64)
    order = np.argsort(-counts, kind="stable")      # global seg ids, size desc
    orig_starts = np.zeros(num_graphs + 1, np.int64)
    np.cumsum(counts, out=orig_starts[1:])

    # snake deal: sorted position p -> (core, local j)
    nloc = num_graphs // n_cores
    pos = np.arange(num_graphs).reshape(nloc, n_cores)
    core_of_pos = np.where((np.arange(nloc) % 2 == 0)[:, None],
                           np.arange(n_cores)[None, :],
                           np.arange(n_cores)[None, :][:, ::-1])
    local_ids = np.empty((n_cores, nloc), np.int64)
    for k in range(n_cores):
        local_ids[k] = order[pos[core_of_pos == k]]
    local_counts = counts[local_ids]                # [n_cores, nloc]

    plans = [_plan(local_counts[k], n_banks) for k in range(n_cores)]
    ngc = [max(len(plans[k][g]) for k in range(n_cores)) for g in range(n_banks)]
    total = sum(ngc)
    ngc[-1] += (-total) % GRP
    nch = sum(ngc)

    G0 = [0]
    for t in ngc:
        G0.append(G0[-1] + t)
    W0 = np.full(nch, np.iinfo(np.int64).max, np.int64)
    W1 = np.full(nch, -1, np.int64)
    for k in range(n_cores):
        for g in range(n_banks):
            for i, (jf, nodes) in enumerate(plans[k][g]):
                c = G0[g] + i
                W0[c] = min(W0[c], jf)
                W1[c] = max(W1[c], nodes[-1][0])
    bank_of = np.empty(nch, np.int64)
    for g in range(n_banks):
        bank_of[G0[g]:G0[g + 1]] = g
        empt = W1[G0[g]:G0[g + 1]] < 0            # all-core-empty pad chunks
        W0[G0[g]:G0[g + 1]][empt] = 512 * g
        W1[G0[g]:G0[g + 1]][empt] = 512 * g
    K = int(max(2, (W1 - W0).max() + 1))
    jb_of = np.minimum(W0 - 512 * bank_of, 512 - K).astype(np.int64)
    assert jb_of.min() >= 0

    sup_sizes = [SUP_CH] * (nch // SUP_CH)
    if nch % SUP_CH:
        sup_sizes.append(nch % SUP_CH)

    # engine splits (tunable): masks on DVE (frac MD) else GPSIMD;
    # psum->sbuf copies: CPAT cycled over GRP-chunk groups (0=DVE, 1=ACT)
    mfrac = float(os.environ.get("MD", "0.3125"))
    mask_dve = [(int(c * mfrac) != int((c + 1) * mfrac)) for c in range(nch)]
    ngrp = nch // GRP
    cpat = [int(v) for v in os.environ.get("CPAT", "0,1").split(",")]
    copy_eng = [cpat[g % len(cpat)] for g in range(ngrp)]

    # per-core tensors
    x_bf = np.ascontiguousarray(x).astype(ml_dtypes.bfloat16)
    in_maps, pad_counts = [], []
    for k in range(n_cores):
        xflat = np.zeros((nch * P, D), ml_dtypes.bfloat16)
        blflat = np.full((nch * P,), 999.0, np.float32)
        real = 0
        for g in range(n_banks):
            for i, (jf, nodes) in enumerate(plans[k][g]):
                c = G0[g] + i
                base = 512 * bank_of[c] + jb_of[c]
                p0 = c * P
                for (j, off, take) in nodes:
                    gid = local_ids[k][j]
                    s0 = orig_starts[gid] + off
                    xflat[p0:p0 + take] = x_bf[s0:s0 + take]
                    blflat[p0:p0 + take] = j - base
                    p0 += take
                    real += take
        pad_counts.append(nch * P - real)
        # slab per super: (c, n, d) -> (d, c, n)
        slabs = []
        o = 0
        for ch in sup_sizes:
            a = xflat[o * P:(o + ch) * P]
            slabs.append(np.ascontiguousarray(
                a.reshape(ch, P, D).transpose(2, 0, 1)).reshape(-1))
            o += ch
        xp = np.concatenate(slabs)
        bl = np.ascontiguousarray(blflat.reshape(nch, P).T).astype(np.float32)
        in_maps.append({
            "xp": xp, "bl": bl,
            "wcol": np.asarray(W, np.float32).reshape(P, 1).astype(ml_dtypes.bfloat16),
            "ident": np.eye(P, dtype=ml_dtypes.bfloat16),
            "iota": np.broadcast_to(
                np.arange(K).astype(ml_dtypes.bfloat16), (P, K)).copy(),
        })

    key = hashlib.sha1(
        np.concatenate([bank_of, jb_of, [nch, K, n_banks]]).tobytes()
        + bytes(mask_dve) + bytes(copy_eng) + bytes(str(sup_sizes), "ascii")
    ).hexdigest()
    if key not in _prog_cache:
        _prog_cache[key] = _build(nch, K, n_banks, bank_of.tolist(),
                                  jb_of.tolist(), sup_sizes, mask_dve, copy_eng)
    nc = _prog_cache[key]

    res = run_bass_kernel_spmd(nc, in_maps, list(range(n_cores))).results

    z_total = 0.0
    out = np.zeros((num_graphs, D), np.float32)
    for k in range(n_cores):
        z_total += float(res[k]["zout"].astype(np.float64).sum()) - pad_counts[k]
        o = res[k]["outp"].astype(np.float32)       # [D, n_banks*512]
        out[local_ids[k]] = o.T[:nloc]
    return (out / np.float32(z_total)).astype(np.float32)


def kernel(x, batch, W, b):
    x = np.asarray(x, np.float32)
    batch = np.asarray(batch).astype(np.int64)
    W = np.asarray(W, np.float32)
    return _pool(x, batch, W, num_graphs=16384)


if __name__ == "__main__":
    rng = np.random.default_rng(0)
    G = int(os.environ.get("TG", "1024"))
    n = int(os.environ.get("TN", "64000"))
    x = rng.standard_normal((n, D), dtype=np.float32)
    batch = np.sort(rng.integers(0, G, n)).astype(np.int64)
    W = (rng.standard_normal((D, 1), dtype=np.float32) / np.sqrt(D)).astype(np.float32)
    b = np.zeros((1,), np.float32)

    got = _pool(x, batch, W, num_graphs=G)

    s = (x @ W).ravel()
    a = np.exp(s - s.max()); a /= a.sum()
    want = np.zeros((G, D), np.float64)
    np.add.at(want, batch, x * a[:, None])
    want = want.astype(np.float32)
    num = np.abs(got - want).max()
    print("abs err:", num, "rel err:", num / np.abs(want).max())
